# revision 1
# baseline (speedup 1.0000x reference)
"""Trainium2 Bass kernel: BertUnpadSelfAttention (B=8, S=1024, H=12, D=64).

Strategy
--------
Data-parallel over batch: core b handles batch b (all 12 heads).

Host prep (per call):
  * scatter unpadded hidden rows into dense [B*S, 768] (+ valid-row flag), like
    the reference's pad_input
  * fold the 1/sqrt(D) score scale into the W/bias q-columns
  * transpose: hT_aug = [hidden_padded | valid]^T  -> [769, 1024] fp16 per core
  * Eb = exp(bias) * 2^-4 transposed to [H, k, q] fp16 per core
    (softmax(s+bias) = (exp(s)*Eb) / sum(exp(s)*Eb); the 2^-4 scale cancels in
    the normalization and keeps products inside fp16 range)

Device (per core), all matmuls fp16 -> fp32 PSUM:
  * v = hT^T @ Wv stored [t, h, d|1] with a ones column appended (the ones
    column yields the softmax denominator through the same matmul chain)
  * per head pair: qT/kT projection [d-on-partitions, t] (so per-head
    scores^T = kT.T @ qT needs no transposes), then per head and k-chunk:
    scores^T [128k, 1024q] (two matmuls, PSUM) -> one exp on ScalarE -> fp16,
    one multiply by the Eb tile on VectorE -> p^T fp16, then
    attnT[d|sum, q-half] += [v|1]^T @ p^T  (one matmul per q-half, a single
    PSUM-bank accumulation group over the 8 k-chunks)
  * attnT tiles evacuate to SBUF fp16 and DMA out as [12, 65, 1024]
  * Eb streams as 96 x [128,1024] fp16 tiles with a deep prefetch pool -- the
    kernel is memory-bound on this stream (25 MB/core)

Host post: out[q, h*64+d] = attnT[h, d, q] / attnT[h, 64, q], gather rows by
`indices` -> (nnz, 768) fp32.
"""

import numpy as np

B, S, H, D = 8, 1024, 12, 64
HID = H * D            # 768
BS = B * S             # 8192
NC = 8                 # cores
EB_SCALE = 0.0625      # folded into exp(bias); cancels in softmax

_CACHE = {}


def _build_nc(reps=1, use_bias=True):
    import concourse.mybir as mybir
    import concourse.tile as tile
    from concourse import bacc

    f16 = mybir.dt.float16

    nc = bacc.Bacc("TRN2", debug=False, num_devices=NC)
    hT = nc.dram_tensor("hT", [769, S], f16, kind="ExternalInput").ap()
    W = nc.dram_tensor("W", [769, 3 * HID], f16, kind="ExternalInput").ap()
    EbT = nc.dram_tensor("EbT", [H, S, S], f16, kind="ExternalInput").ap()
    out = nc.dram_tensor("out", [H, D + 1, S], f16, kind="ExternalOutput").ap()

    with tile.TileContext(nc) as tc:
        for _ in range(reps):
            _emit_body(nc, tc, tile, mybir, hT, W, EbT, out, use_bias)
    nc.compile()
    return nc


def _emit_body(nc, tc, tile, mybir, hT, W, EbT, out, use_bias):
    f16 = mybir.dt.float16
    f32 = mybir.dt.float32
    Exp = mybir.ActivationFunctionType.Exp
    with (
        tc.tile_pool(name="per", bufs=1) as per,
        tc.tile_pool(name="ebp", bufs=24) as ebp,
        tc.tile_pool(name="st", bufs=8) as st,
        tc.tile_pool(name="psm", bufs=3, space="PSUM") as psm,
        tc.tile_pool(name="pat", bufs=4, space="PSUM") as pat,
        tc.tile_pool(name="pj", bufs=1, space="PSUM") as pjp,
    ):
        # ---- persistent loads -------------------------------------------
        hT_sb = per.tile([128, 6, S], f16)
        for ic in range(6):
            nc.sync.dma_start(hT_sb[:, ic], hT[ic * 128:(ic + 1) * 128, :])
        W_sb = per.tile([128, 6, 3 * HID], f16)
        for ic in range(6):
            nc.sync.dma_start(W_sb[:, ic], W[ic * 128:(ic + 1) * 128, :])
        if use_bias:
            hT_last = per.tile([1, S], f16)
            nc.sync.dma_start(hT_last, hT[768:769, :])
            W_last = per.tile([1, 3 * HID], f16)
            nc.sync.dma_start(W_last, W[768:769, :])

        # q^T/k^T: [128 = head-pair d dims, pair, t]; head 2p+half lives on
        # partitions half*64..half*64+63 of pair p
        qT_sb = per.tile([128, 6, S], f16)
        kT_sb = per.tile([128, 6, S], f16)
        # v with ones column: [t_in_chunk, t_chunk, head, d|1]
        vv = per.tile([128, 8, H, D + 1], f16)
        out_sb = per.tile([65, H, S], f16)
        nc.vector.memset(vv, 1.0)

        # ---- projection jobs (interleaved into the attention k-loop) ----
        # v projection at (t-chunk, head-pair) granularity: N=128 columns
        def v_job(t8, vp):
            def run():
                n0 = vp * 128
                ps = psm.tile([128, 128], f32, tag="mm", name="ps_v",
                              padded_shape=[128, 512])
                for ic in range(6):
                    nc.tensor.matmul(
                        ps,
                        hT_sb[:, ic, t8 * 128:(t8 + 1) * 128],
                        W_sb[:, ic, 2 * HID + n0:2 * HID + n0 + 128],
                        start=(ic == 0), stop=(not use_bias and ic == 5),
                    )
                if use_bias:
                    nc.tensor.matmul(
                        ps,
                        hT_last[:, t8 * 128:(t8 + 1) * 128],
                        W_last[:, 2 * HID + n0:2 * HID + n0 + 128],
                        start=False, stop=True,
                    )
                nc.vector.tensor_copy(
                    vv[:, t8, 2 * vp:2 * vp + 2, 0:D],
                    ps.rearrange("p (h d) -> p h d", d=D),
                )
            return run

        def qk_half(pair, ci, t2, lo):
            # half of a q/k projection tile (3 of the 6 K-chunks)
            def run(ps):
                ics = range(0, 3) if lo else range(3, 6)
                for ic in ics:
                    nc.tensor.matmul(
                        ps,
                        W_sb[:, ic, ci * 128:(ci + 1) * 128],
                        hT_sb[:, ic, t2 * 512:(t2 + 1) * 512],
                        start=(ic == 0), stop=(not use_bias and ic == 5),
                    )
                if not lo:
                    if use_bias:
                        nc.tensor.matmul(
                            ps,
                            W_last[:, ci * 128:(ci + 1) * 128],
                            hT_last[:, t2 * 512:(t2 + 1) * 512],
                            start=False, stop=True,
                        )
                    dest = qT_sb if ci < 6 else kT_sb
                    nc.vector.tensor_copy(dest[:, pair, t2 * 512:(t2 + 1) * 512], ps)
            return run

        def qk_jobs(pair):
            # 8 half-jobs; consecutive pairs share a psum tile via a box
            jobs = []
            for ci in (pair, 6 + pair):
                for t2 in range(2):
                    box = {}
                    def mk(fn, box, first):
                        def run():
                            if first:
                                box["ps"] = pjp.tile([128, 512], f32, tag="qk",
                                                     name="ps_qk")
                            fn(box["ps"])
                        return run
                    jobs.append(mk(qk_half(pair, ci, t2, True), box, True))
                    jobs.append(mk(qk_half(pair, ci, t2, False), box, False))
            return jobs

        # upfront: pair-0 q/k and its first v chunk
        for job in qk_jobs(0):
            job()
        v_job(0, 0)()

        # ---- attention: per head pair, k-chunk loop ---------------------
        # v chunks stream just ahead of their consuming k-step; pair p+1's
        # q/k half-jobs stream through pair p's k-loop
        for pair in range(6):
            sched = {kc: [] for kc in range(8)}
            for t8 in range(8):
                # vv[t8, pair p'] for the NEXT pair (p'=pair+1) ahead of time;
                # pair 0 also needs its own chunks t8>=1 just in time
                if pair == 0 and t8 >= 1:
                    sched[t8 - 1].append(v_job(t8, 0))
                if pair < 5:
                    sched[t8].append(v_job(t8, pair + 1))
            if pair < 5:
                for i, job in enumerate(qk_jobs(pair + 1)):
                    sched[i].append(job)

            att = {
                half: [pat.tile([D + 1, 512], f32, tag="at", name="att")
                       for _ in range(2)]
                for half in range(2)
            }

            def emit_pv(pts, kc):
                for half in range(2):
                    for qc in range(2):
                        nc.tensor.matmul(
                            att[half][qc],
                            vv[:, kc, 2 * pair + half, :],
                            pts[(half, qc)],
                            start=(kc == 0), stop=(kc == 7),
                        )

            prev = None
            for kc in range(8):
                ebs = []
                for half in range(2):
                    eb = ebp.tile([128, S], f16, tag="eb", name="eb")
                    nc.sync.dma_start(
                        eb, EbT[2 * pair + half, kc * 128:(kc + 1) * 128, :]
                    )
                    ebs.append(eb)
                spss = {}
                # interleave the two heads' score matmuls: disjoint PE
                # row-groups (partitions 0-63 vs 64-127) run concurrently
                for qc in range(2):
                    for half in range(2):
                        p0 = half * 64
                        sps = psm.tile([128, 512], f32, tag="mm", name="sps")
                        nc.tensor.matmul(
                            sps,
                            kT_sb[p0:p0 + 64, pair, kc * 128:(kc + 1) * 128],
                            qT_sb[p0:p0 + 64, pair, qc * 512:(qc + 1) * 512],
                            start=True, stop=True,
                        )
                        spss[(half, qc)] = sps
                # software pipeline: PE runs the previous k-chunk's p@v and
                # interleaved projection work while ScalarE/VectorE produce
                # this chunk's p^T
                if prev is not None:
                    emit_pv(*prev)
                for job in sched[kc]:
                    job()
                pts = {}
                for half in range(2):
                    for qc in range(2):
                        es = st.tile([128, 512], f16, tag="es", name="es")
                        nc.scalar.activation(es, spss[(half, qc)], Exp)
                        pt = st.tile([128, 512], f16, tag="pt", name="pt")
                        nc.vector.tensor_mul(
                            pt, es, ebs[half][:, qc * 512:(qc + 1) * 512]
                        )
                        pts[(half, qc)] = pt
                prev = (pts, kc)
            emit_pv(*prev)
            for half in range(2):
                for qc in range(2):
                    nc.vector.tensor_copy(
                        out_sb[:, 2 * pair + half, qc * 512:(qc + 1) * 512],
                        att[half][qc],
                    )

        # ---- store: out[h, d|sum, q] ------------------------------------
        for h in range(H):
            nc.sync.dma_start(out[h], out_sb[:, h, :])


def _get_nc(use_bias=True):
    key = ("nc", use_bias)
    if key not in _CACHE:
        _CACHE[key] = _build_nc(use_bias=use_bias)
    return _CACHE[key]


def prepare_in_maps(inputs):
    """Host-side shard/prep: returns (in_maps for 8 cores, indices, use_bias)."""
    hidden = np.asarray(inputs["hidden_states"], np.float32)
    W = np.array(np.asarray(inputs["Wqkv_w"], np.float32))
    b = np.array(np.asarray(inputs["Wqkv_b"], np.float32))
    bias = np.asarray(inputs["bias"], np.float32)
    indices = np.asarray(inputs["indices"], np.int32)
    use_bias = bool(np.any(b != 0.0))

    scale = 1.0 / np.sqrt(np.float32(D))
    Ws = W.copy()
    Ws[:, :HID] *= scale
    bs = b.copy()
    bs[:HID] *= scale
    W_aug = np.concatenate([Ws, bs[None, :]], axis=0).astype(np.float16)

    hp = np.zeros((BS, HID), np.float32)
    hp[indices] = hidden
    valid = np.zeros((1, BS), np.float32)
    valid[0, indices] = 1.0

    def prep_core(c):
        hTa = np.concatenate(
            [hp[c * S:(c + 1) * S].T, valid[:, c * S:(c + 1) * S]], axis=0
        ).astype(np.float16)
        ebt = np.empty((H, S, S), np.float16)
        for h in range(H):
            ebt[h] = (np.exp(bias[c, h]) * EB_SCALE).T.astype(np.float16)
        return {"hT": hTa, "W": W_aug, "EbT": ebt}

    from concurrent.futures import ThreadPoolExecutor
    with ThreadPoolExecutor(max_workers=8) as ex:
        in_maps = list(ex.map(prep_core, range(NC)))
    return in_maps, indices, use_bias


def postprocess(results, indices):
    """results[c]['out'] is [H, D+1, S] fp16; divide, transpose, gather."""
    full = np.empty((BS, HID), np.float32)
    for c in range(NC):
        a = np.asarray(results[c]["out"], np.float32)      # [H, 65, S]
        r = a[:, :D, :] / a[:, D:D + 1, :]                 # [H, D, S]
        full[c * S:(c + 1) * S] = r.transpose(2, 0, 1).reshape(S, HID)
    return full[indices]


def _run_spmd(in_maps, use_bias=True, trace=False):
    from concourse.bass_utils import run_bass_kernel_spmd
    return run_bass_kernel_spmd(
        _get_nc(use_bias), in_maps, core_ids=list(range(NC)), trace=trace
    )


def kernel(**inputs):
    in_maps, indices, use_bias = prepare_in_maps(inputs)
    res = _run_spmd(in_maps, use_bias=use_bias)
    return postprocess(res.results, indices)



# revision 14
# speedup vs baseline: 1.0219x; 1.0219x over previous
"""Trainium2 Bass kernel: BertUnpadSelfAttention (B=8, S=1024, H=12, D=64).

Strategy (v2)
-------------
Shard by (batch, head) pairs instead of batch: the sequence lengths vary
(512..1024), so batch-per-core sharding makes every core pay for the
longest batch.  96 (b,h) pairs are grouped into length-classes and
bin-packed into an identical per-core slot schedule (same instruction
stream on all 8 cores; the host packs each core's pairs into the slots).
Masked work is skipped entirely: k-chunks and q-chunks beyond L are never
computed and their exp(bias) tiles are never streamed.

Device (per core, fp16 matmuls -> fp32 PSUM):
  * proj per pair: qk^T = W_qk^T @ hT -> [128 = q64|k64 dims, L tokens];
    v = hT^T @ W_v -> [token, head, d] with a ones column for the softmax
    denominator
  * per slot (pair, q-chunk<=512), per k-chunk of 128:
    scores^T [128k, qlen] = kT.T @ qT (one matmul), exp on ScalarE,
    multiply by host-precomputed exp(bias)*2^-4 tile on VectorE,
    then reverse-PV: att[q<=128, d|sum] += p^T.T @ [v|1]  (probs as the
    stationary operand: 65-row streams instead of 512 -> half the PE time)
  * proj of the next class is interleaved into the attention k-loop;
    PSUM->SBUF evacuations run on GpSimd (otherwise idle)
  * Eb streams via few large descriptor-rich DMAs issued from GpSimd

Host: pack per-core hT/W/Eb; post: divide by denominator, write rows
directly at cu_seqlens offsets (valid tokens are contiguous per batch).
"""

import numpy as np

B, S, H, D = 8, 1024, 12, 64
HID = H * D            # 768
NC = 8                 # cores
KCH = 128              # k chunk
QCH = 512              # max q chunk / moving free dim
EB_SCALE = 0.0625      # folded into exp(bias); cancels in softmax

_CACHE = {}


# --------------------------------------------------------------------------
# schedule
# --------------------------------------------------------------------------

class _Cls:
    __slots__ = ("L", "Lp", "nk", "np_", "ngrp", "ht_off", "vv_base",
                 "qk_offs", "w_blocks", "qslots", "pairs_percore")


class _Slot:
    __slots__ = ("ci", "pl", "qoff", "qlen", "nsub", "eb_row", "out_off")


def _build_schedule(lens):
    """Uniform per-core slot schedule from the 8 sequence lengths."""
    lens = [int(x) for x in lens]
    assert len(lens) == B and all(0 < l <= S for l in lens)

    # merge batches with equal L into one class; if a core's slice of a
    # class would straddle two batches, fall back to one class per batch
    def mk_classes(group_by_len):
        if group_by_len:
            ls = sorted(set(lens), reverse=True)
            return [(L, [b for b in range(B) if lens[b] == L]) for L in ls]
        order = sorted(range(B), key=lambda b: -lens[b])
        return [(lens[b], [b]) for b in order]

    for group_by_len in (True, False):
        classes = []
        ok = True
        for L, batches in mk_classes(group_by_len):
            pairs = [(b, h) for b in batches for h in range(H)]
            np_ = -(-len(pairs) // NC)
            padded = pairs + [None] * (np_ * NC - len(pairs))
            percore = [padded[c * np_:(c + 1) * np_] for c in range(NC)]
            for c in range(NC):
                bs = {p[0] for p in percore[c] if p is not None}
                if len(bs) > 1:
                    ok = False
            classes.append((L, percore, np_))
        if ok:
            break
    assert ok, "schedule fallback failed"

    # W group-block dedupe: pairs are processed in groups of 2 (two swapped
    # projection passes [qA|kB], [kA|qB] so both operands of the scores
    # matmul share a base partition).  A group block can be reused if on
    # every core the (headA, headB) at that block matches (or is pad).
    block_heads = [dict() for _ in range(NC)]   # per core: block -> (hA, hB)
    nblocks = 0
    out_classes = []
    ht_off = vv_base = qk_off = 0
    for L, percore, np_ in classes:
        nk = -(-L // KCH)
        Lp = nk * KCH
        ngrp = -(-np_ // 2)
        cls = _Cls()
        cls.L, cls.Lp, cls.nk, cls.np_, cls.ngrp = L, Lp, nk, np_, ngrp
        cls.pairs_percore = percore
        cls.ht_off = ht_off
        cls.vv_base = vv_base
        cls.qk_offs = []
        cls.w_blocks = []

        def _grp_heads(c, g):
            pa = percore[c][2 * g]
            pb = percore[c][2 * g + 1] if 2 * g + 1 < np_ else None
            return (pa[1] if pa is not None else None,
                    pb[1] if pb is not None else None)

        for g in range(ngrp):
            hs = [_grp_heads(c, g) for c in range(NC)]

            def _compat(cand):
                for c, (ha, hb) in enumerate(hs):
                    cur = block_heads[c].get(cand)
                    if cur is not None:
                        if ha is not None and cur[0] is not None and cur[0] != ha:
                            return False
                        if hb is not None and cur[1] is not None and cur[1] != hb:
                            return False
                return True

            beta = None
            for cand in range(nblocks):
                if _compat(cand):
                    beta = cand
                    break
            if beta is None:
                beta = nblocks
                nblocks += 1
            for c, (ha, hb) in enumerate(hs):
                cur = block_heads[c].get(beta, (None, None))
                block_heads[c][beta] = (ha if ha is not None else cur[0],
                                        hb if hb is not None else cur[1])
            cls.w_blocks.append(beta)
            cls.qk_offs.append((qk_off, qk_off + Lp))
            qk_off += 2 * Lp
        # v-proj streams the class's blocks as one range: need contiguity
        b0 = cls.w_blocks[0]
        if cls.w_blocks != list(range(b0, b0 + ngrp)):
            cls.w_blocks = list(range(nblocks, nblocks + ngrp))
            nblocks += ngrp
            for g in range(ngrp):
                for c in range(NC):
                    block_heads[c][cls.w_blocks[g]] = _grp_heads(c, g)
        cls.qslots = []
        q = 0
        while q < L:
            cls.qslots.append((q, min(QCH, L - q)))
            q += QCH
        ht_off += Lp
        vv_base += nk * np_
        out_classes.append(cls)

    slots = []
    eb_row = out_off = 0
    for ci, cls in enumerate(out_classes):
        for pl in range(cls.np_):
            for (qoff, qlen) in cls.qslots:
                sl = _Slot()
                sl.ci, sl.pl, sl.qoff, sl.qlen = ci, pl, qoff, qlen
                sl.nsub = -(-qlen // 128)
                sl.eb_row = eb_row
                sl.out_off = out_off
                eb_row += cls.nk * KCH
                out_off += sl.nsub
                slots.append(sl)

    sched = {
        "classes": out_classes,
        "slots": slots,
        "NW": nblocks,
        "HT_COLS": ht_off,
        "QKT_COLS": qk_off,
        "VCH": vv_base,
        "EB_ROWS": eb_row,
        "NSUB": out_off,
        "lens": lens,
    }
    return sched


def _get_sched(lens_key):
    key = ("sched", lens_key)
    if key not in _CACHE:
        _CACHE[key] = _build_schedule(list(lens_key))
    return _CACHE[key]


# --------------------------------------------------------------------------
# device program
# --------------------------------------------------------------------------

def _build_nc(sched, use_bias):
    import concourse.mybir as mybir
    import concourse.tile as tile
    from concourse import bacc

    f16 = mybir.dt.float16

    nc = bacc.Bacc("TRN2", debug=False, num_devices=NC)
    hT = nc.dram_tensor("hT", [HID + 1, sched["HT_COLS"]], f16,
                        kind="ExternalInput").ap()
    W = nc.dram_tensor("W", [HID + 1, sched["NW"] * 384], f16,
                       kind="ExternalInput").ap()
    Eb = nc.dram_tensor("Eb", [sched["EB_ROWS"], QCH], f16,
                        kind="ExternalInput").ap()
    out = nc.dram_tensor("out", [sched["NSUB"], 128, 65], f16,
                         kind="ExternalOutput").ap()

    with tile.TileContext(nc) as tc:
        _emit_body(nc, tc, tile, mybir, hT, W, Eb, out, sched, use_bias)
    nc.compile()
    return nc


def _emit_body(nc, tc, tile, mybir, hT, W, Eb, out, sched, use_bias):
    f16 = mybir.dt.float16
    f32 = mybir.dt.float32
    Exp = mybir.ActivationFunctionType.Exp
    classes = sched["classes"]
    slots = sched["slots"]
    NW = sched["NW"]

    with (
        tc.tile_pool(name="per", bufs=1) as per,
        tc.tile_pool(name="ebp", bufs=3) as ebp,
        tc.tile_pool(name="st", bufs=18) as st,
        tc.tile_pool(name="osb", bufs=3) as osb,
        tc.tile_pool(name="psc", bufs=2, space="PSUM") as psc,
        tc.tile_pool(name="pat", bufs=2, space="PSUM") as pat,
        tc.tile_pool(name="pjq", bufs=2, space="PSUM") as pjq,
        tc.tile_pool(name="pjv", bufs=2, space="PSUM") as pjv,
    ):
        # ---- persistent tiles ------------------------------------------
        # W group block (384 cols): [qA|kB][kA|qB][vA|vB]
        hT_sb = per.tile([128, 6, sched["HT_COLS"]], f16)
        W_sb = per.tile([128, 6, NW, 384], f16)
        qkT = per.tile([128, sched["QKT_COLS"]], f16)
        vv = per.tile([128, sched["VCH"], 65], f16)
        nc.vector.memset(vv[:, :, 64:65], 1.0)
        if use_bias:
            hT_last = per.tile([1, sched["HT_COLS"]], f16)
            W_last = per.tile([1, NW, 384], f16)
            nc.sync.dma_start(hT_last, hT[HID:HID + 1, :])
            nc.sync.dma_start(
                W_last, W[HID:HID + 1, :].rearrange("o (w k) -> o w k", k=384)
            )

        # ---- upfront DMAs ----------------------------------------------
        nc.sync.dma_start(
            W_sb,
            W[0:HID, :].rearrange("(i p) (w k) -> p i w k", p=128, k=384),
        )
        for cls in classes:
            nc.sync.dma_start(
                hT_sb[:, :, cls.ht_off:cls.ht_off + cls.Lp],
                hT[0:HID, cls.ht_off:cls.ht_off + cls.Lp].rearrange(
                    "(i p) c -> p i c", p=128
                ),
            )

        # ---- projection job closures -----------------------------------
        def qk_job(cls, g, pss, lc):
            # pss 0: W cols [0:128] = [qA|kB]; pss 1: [128:256] = [kA|qB]
            def run():
                cw = min(QCH, cls.Lp - lc * QCH)
                beta = cls.w_blocks[g]
                c0 = 128 * pss
                dst = cls.qk_offs[g][pss] + lc * QCH
                ps = pjq.tile([128, QCH], f32, tag="qk", name="ps_qk")
                for ic in range(6):
                    nc.tensor.matmul(
                        ps[:, :cw],
                        W_sb[:, ic, beta, c0:c0 + 128],
                        hT_sb[:, ic, cls.ht_off + lc * QCH:
                              cls.ht_off + lc * QCH + cw],
                        start=(ic == 0), stop=(ic == 5 and not use_bias),
                    )
                if use_bias:
                    nc.tensor.matmul(
                        ps[:, :cw],
                        W_last[:, beta, c0:c0 + 128],
                        hT_last[:, cls.ht_off + lc * QCH:
                                cls.ht_off + lc * QCH + cw],
                        start=False, stop=True,
                    )
                nc.vector.tensor_copy(qkT[:, dst:dst + cw], ps[:, :cw])
            return run

        def v_job(cls, kc):
            # moving = v cols of the class's group blocks; the last group's
            # second half is skipped when np_ is odd (two matmul chains)
            def run():
                npr = cls.np_
                b0 = cls.w_blocks[0]
                nfull = npr // 2
                ps = pjv.tile([128, QCH], f32, tag="v", name="ps_v")
                segs = []
                if nfull:
                    segs.append((0, W_sb[:, :, b0:b0 + nfull, 256:384],
                                 nfull * 128,
                                 (W_last[:, b0:b0 + nfull, 256:384]
                                  if use_bias else None)))
                if npr % 2:
                    gl = b0 + nfull
                    segs.append((nfull * 128, W_sb[:, :, gl, 256:320], 64,
                                 (W_last[:, gl, 256:320]
                                  if use_bias else None)))
                for (o0, wap, n, wlast) in segs:
                    for ic in range(6):
                        nc.tensor.matmul(
                            ps[:, o0:o0 + n],
                            hT_sb[:, ic, cls.ht_off + kc * KCH:
                                  cls.ht_off + (kc + 1) * KCH],
                            wap[:, ic],
                            start=(ic == 0), stop=(ic == 5 and not use_bias),
                        )
                    if use_bias:
                        nc.tensor.matmul(
                            ps[:, o0:o0 + n],
                            hT_last[:, cls.ht_off + kc * KCH:
                                    cls.ht_off + (kc + 1) * KCH],
                            wlast,
                            start=False, stop=True,
                        )
                nc.vector.tensor_copy(
                    vv[:, cls.vv_base + kc * npr:
                       cls.vv_base + (kc + 1) * npr, 0:64],
                    ps[:, :npr * 64].rearrange("p (h d) -> p h d", d=64),
                )
            return run

        def proj_jobs(ci):
            cls = classes[ci]
            jobs = []
            for g in range(cls.ngrp):
                for pss in range(2):
                    for lc in range(-(-cls.Lp // QCH)):
                        jobs.append(qk_job(cls, g, pss, lc))
            for kc in range(cls.nk):
                jobs.append(v_job(cls, kc))
            return jobs

        # ---- Eb prefetch ------------------------------------------------
        eb_tiles = {}

        def issue_eb(si):
            sl = slots[si]
            cls = classes[sl.ci]
            t = ebp.tile([128, 8, QCH], f16, tag="eb", name="eb")
            eb_tiles[si] = t
            for kg in range(0, cls.nk, 2):
                n = min(2, cls.nk - kg)
                r0 = sl.eb_row + kg * KCH
                nc.gpsimd.dma_start(
                    t[:, kg:kg + n, :sl.qlen],
                    Eb[r0:r0 + n * KCH, 0:sl.qlen].rearrange(
                        "(n p) q -> p n q", p=128
                    ),
                )

        # ---- prologue ---------------------------------------------------
        for job in proj_jobs(0):
            job()
        issue_eb(0)
        if len(slots) > 1:
            issue_eb(1)

        # ---- main loop --------------------------------------------------
        # software pipeline: scores/exp/mul for slot i run while the PV
        # matmuls for slot i-1 stream (each q-sub's PSUM accumulation group
        # is sequential in its own bank: start=True resets the whole bank)
        def emit_pv_all(pend):
            (pts, cls_p, pl_p, qlen_p, nsub_p, out_off_p) = pend
            ob = osb.tile([128, 4, 65], f16, tag="ob", name="ob")
            for sub in range(nsub_p):
                qn = min(128, qlen_p - sub * 128)
                att = pat.tile([128, 128], f32, tag="att", name="att",
                               padded_shape=[128, QCH])
                for kc in range(cls_p.nk):
                    nc.tensor.matmul(
                        att[0:qn, 0:65],
                        pts[kc][:, sub * 128:sub * 128 + qn],
                        vv[:, cls_p.vv_base + kc * cls_p.np_ + pl_p, :],
                        start=(kc == 0), stop=(kc == cls_p.nk - 1),
                    )
                nc.vector.tensor_copy(ob[:, sub, :], att[:, 0:65])
            nc.sync.dma_start(
                out[out_off_p:out_off_p + nsub_p].rearrange("n p x -> p n x"),
                ob[:, :nsub_p, :],
            )

        si = 0
        pending = None
        for ci, cls in enumerate(classes):
            fillers = proj_jobs(ci + 1) if ci + 1 < len(classes) else []
            n_iters = cls.np_ * len(cls.qslots) * cls.nk
            stride = max(1, n_iters // max(1, len(fillers)))
            it = 0
            fi = 0
            for pl in range(cls.np_):
                for (qoff, qlen) in cls.qslots:
                    sl = slots[si]
                    eb = eb_tiles.pop(si)
                    g, half = pl // 2, pl % 2
                    off1, off2 = cls.qk_offs[g]
                    # half 0: q in T1[0:64],  k in T2[0:64]
                    # half 1: q in T2[64:128], k in T1[64:128]
                    p0 = 64 * half
                    koff = off1 if half else off2
                    qoff_t = off2 if half else off1

                    pts = []
                    for kc in range(cls.nk):
                        sps = psc.tile([128, QCH], f32, tag="sc", name="sps")
                        nc.tensor.matmul(
                            sps[:, :qlen],
                            qkT[p0:p0 + 64,
                                koff + kc * KCH:koff + (kc + 1) * KCH],
                            qkT[p0:p0 + 64,
                                qoff_t + qoff:qoff_t + qoff + qlen],
                            start=True, stop=True,
                        )
                        if fillers and fi < len(fillers) and it % stride == 0:
                            fillers[fi]()
                            fi += 1
                        it += 1
                        es = st.tile([128, QCH], f16, tag="es", name="es")
                        nc.scalar.activation(es[:, :qlen], sps[:, :qlen], Exp)
                        pt = st.tile([128, QCH], f16, tag="pt", name="pt")
                        mul_eng = nc.gpsimd if kc % 3 == 2 else nc.vector
                        mul_eng.tensor_mul(
                            pt[:, :qlen], es[:, :qlen], eb[:, kc, :qlen]
                        )
                        pts.append(pt)
                    if pending is not None:
                        emit_pv_all(pending)
                    pending = (pts, cls, pl, qlen, sl.nsub, sl.out_off)
                    if si + 2 < len(slots):
                        issue_eb(si + 2)
                    si += 1
            while fi < len(fillers):
                fillers[fi]()
                fi += 1
        emit_pv_all(pending)


def _get_nc(lens_key, use_bias):
    key = ("nc", lens_key, use_bias)
    if key not in _CACHE:
        _CACHE[key] = _build_nc(_get_sched(lens_key), use_bias)
    return _CACHE[key]


# --------------------------------------------------------------------------
# host pack / unpack
# --------------------------------------------------------------------------

def prepare_in_maps(inputs):
    hidden = np.asarray(inputs["hidden_states"], np.float32)
    Wf = np.asarray(inputs["Wqkv_w"], np.float32)
    bvec = np.asarray(inputs["Wqkv_b"], np.float32)
    bias = np.asarray(inputs["bias"], np.float32)
    indices = np.asarray(inputs["indices"], np.int32)
    cu = np.asarray(inputs["cu_seqlens"], np.int64)
    lens = np.diff(cu).astype(np.int64)
    nnz = hidden.shape[0]

    # valid tokens must be the first L of each batch row-block
    expect = np.concatenate(
        [b * S + np.arange(l) for b, l in enumerate(lens)]
    ) if len(lens) == B else None
    contiguous = (
        expect is not None
        and indices.shape[0] == expect.shape[0]
        and np.array_equal(indices, expect)
    )
    if not contiguous:
        # fallback: dense full-length processing, scatter rows
        lens = np.full(B, S, np.int64)
        hp = np.zeros((B * S, HID), np.float32)
        hp[indices] = hidden
        tok = [hp[b * S:(b + 1) * S] for b in range(B)]
    else:
        tok = [hidden[cu[b]:cu[b + 1]] for b in range(B)]

    lens_key = tuple(int(x) for x in lens)
    sched = _get_sched(lens_key)
    use_bias = bool(np.any(bvec != 0.0))

    Ws = Wf.copy()
    Ws[:, :HID] *= 0.125          # fold 1/sqrt(D) into q
    bs = bvec.copy()
    bs[:HID] *= 0.125

    classes = sched["classes"]
    slots = sched["slots"]
    NW = sched["NW"]

    def prep_core(c):
        hTa = np.zeros((HID + 1, sched["HT_COLS"]), np.float16)
        hTa[HID] = 1.0
        Wd = np.zeros((HID + 1, NW * 384), np.float16)
        Ebd = np.zeros((sched["EB_ROWS"], QCH), np.float16)
        for cls in classes:
            batches = {p[0] for p in cls.pairs_percore[c] if p is not None}
            if batches:
                b0 = next(iter(batches))
                L = int(lens[b0])
                hTa[0:HID, cls.ht_off:cls.ht_off + L] = tok[b0].T
            for pl, pair in enumerate(cls.pairs_percore[c]):
                if pair is None:
                    continue
                _, h = pair
                beta = cls.w_blocks[pl // 2]
                half = pl % 2
                # group block: [qA|kB][kA|qB][vA|vB]
                qc0 = beta * 384 + (192 if half else 0)
                kc0 = beta * 384 + (64 if half else 128)
                vc0 = beta * 384 + (320 if half else 256)
                Wd[0:HID, qc0:qc0 + 64] = Ws[:, h * D:(h + 1) * D]
                Wd[0:HID, kc0:kc0 + 64] = Ws[:, HID + h * D:HID + (h + 1) * D]
                Wd[0:HID, vc0:vc0 + 64] = \
                    Ws[:, 2 * HID + h * D:2 * HID + (h + 1) * D]
                Wd[HID, qc0:qc0 + 64] = bs[h * D:(h + 1) * D]
                Wd[HID, kc0:kc0 + 64] = bs[HID + h * D:HID + (h + 1) * D]
                Wd[HID, vc0:vc0 + 64] = \
                    bs[2 * HID + h * D:2 * HID + (h + 1) * D]
        with np.errstate(under="ignore"):
            for sl in slots:
                cls = classes[sl.ci]
                pair = cls.pairs_percore[c][sl.pl]
                if pair is None:
                    continue
                b, h = pair
                L = int(lens[b])
                sub = bias[b, h, sl.qoff:sl.qoff + sl.qlen, 0:L]
                Ebd[sl.eb_row:sl.eb_row + L, :sl.qlen] = (
                    (np.exp(sub) * EB_SCALE).T.astype(np.float16)
                )
        return {"hT": hTa, "W": Wd, "Eb": Ebd}

    from concurrent.futures import ThreadPoolExecutor
    with ThreadPoolExecutor(max_workers=8) as ex:
        in_maps = list(ex.map(prep_core, range(NC)))

    meta = {
        "lens_key": lens_key,
        "cu": cu,
        "nnz": nnz,
        "contiguous": contiguous,
        "indices": indices,
    }
    return in_maps, meta, use_bias


def postprocess(results, meta):
    sched = _get_sched(meta["lens_key"])
    classes = sched["classes"]
    slots = sched["slots"]
    cu = meta["cu"]
    if meta["contiguous"]:
        out_full = np.zeros((meta["nnz"], HID), np.float32)
    else:
        out_full = np.zeros((B * S, HID), np.float32)
    for c in range(NC):
        o = np.asarray(results[c]["out"], np.float32)   # [NSUB, 128, 65]
        for sl in slots:
            cls = classes[sl.ci]
            pair = cls.pairs_percore[c][sl.pl]
            if pair is None:
                continue
            b, h = pair
            base = (cu[b] if meta["contiguous"] else b * S)
            for sub in range(sl.nsub):
                qn = min(128, sl.qlen - sub * 128)
                blk = o[sl.out_off + sub, :qn]
                att = blk[:, :64] / blk[:, 64:65]
                r0 = base + sl.qoff + sub * 128
                out_full[r0:r0 + qn, h * D:(h + 1) * D] = att
    if not meta["contiguous"]:
        out_full = out_full[meta["indices"]]
    return out_full


def _run_spmd(in_maps, meta, use_bias=True, trace=False):
    from concourse.bass_utils import run_bass_kernel_spmd
    return run_bass_kernel_spmd(
        _get_nc(meta["lens_key"], use_bias), in_maps,
        core_ids=list(range(NC)), trace=trace,
    )


def kernel(**inputs):
    in_maps, meta, use_bias = prepare_in_maps(inputs)
    res = _run_spmd(in_maps, meta, use_bias=use_bias)
    return postprocess(res.results, meta)


# revision 19
# speedup vs baseline: 1.3436x; 1.3148x over previous
"""Trainium2 Bass kernel: BertUnpadSelfAttention (B=8, S=1024, H=12, D=64).

Strategy (v2)
-------------
Shard by (batch, head) pairs instead of batch: the sequence lengths vary
(512..1024), so batch-per-core sharding makes every core pay for the
longest batch.  96 (b,h) pairs are grouped into length-classes and
bin-packed into an identical per-core slot schedule (same instruction
stream on all 8 cores; the host packs each core's pairs into the slots).
Masked work is skipped entirely: k-chunks and q-chunks beyond L are never
computed and their exp(bias) tiles are never streamed.

Device (per core, fp16 matmuls -> fp32 PSUM):
  * proj per pair: qk^T = W_qk^T @ hT -> [128 = q64|k64 dims, L tokens];
    v = hT^T @ W_v -> [token, head, d] with a ones column for the softmax
    denominator
  * per slot (pair, q-chunk<=512), per k-chunk of 128:
    scores^T [128k, qlen] = kT.T @ qT (one matmul), exp on ScalarE,
    multiply by host-precomputed exp(bias)*2^-4 tile on VectorE,
    then reverse-PV: att[q<=128, d|sum] += p^T.T @ [v|1]  (probs as the
    stationary operand: 65-row streams instead of 512 -> half the PE time)
  * proj of the next class is interleaved into the attention k-loop;
    PSUM->SBUF evacuations run on GpSimd (otherwise idle)
  * Eb streams via few large descriptor-rich DMAs issued from GpSimd

Host: pack per-core hT/W/Eb; post: divide by denominator, write rows
directly at cu_seqlens offsets (valid tokens are contiguous per batch).
"""

import numpy as np

B, S, H, D = 8, 1024, 12, 64
HID = H * D            # 768
NC = 8                 # cores
KCH = 128              # k chunk
QCH = 512              # max q chunk / moving free dim
EB_SCALE = 0.0625      # folded into exp(bias); cancels in softmax

_CACHE = {}


# --------------------------------------------------------------------------
# schedule
# --------------------------------------------------------------------------

class _Cls:
    __slots__ = ("L", "Lp", "nk", "np_", "ngrp", "ht_off", "vv_base",
                 "qk_offs", "w_blocks", "qslots", "pairs_percore")


class _Slot:
    __slots__ = ("ci", "pl", "qoff", "qlen", "nsub", "eb_col", "out_off")


def _build_schedule(lens):
    """Uniform per-core slot schedule from the 8 sequence lengths."""
    lens = [int(x) for x in lens]
    assert len(lens) == B and all(0 < l <= S for l in lens)

    # merge batches with equal L into one class; if a core's slice of a
    # class would straddle two batches, fall back to one class per batch
    def mk_classes(group_by_len):
        if group_by_len:
            ls = sorted(set(lens), reverse=True)
            return [(L, [b for b in range(B) if lens[b] == L]) for L in ls]
        order = sorted(range(B), key=lambda b: -lens[b])
        return [(lens[b], [b]) for b in order]

    for group_by_len in (True, False):
        classes = []
        ok = True
        for L, batches in mk_classes(group_by_len):
            pairs = [(b, h) for b in batches for h in range(H)]
            np_ = -(-len(pairs) // NC)
            padded = pairs + [None] * (np_ * NC - len(pairs))
            percore = [padded[c * np_:(c + 1) * np_] for c in range(NC)]
            for c in range(NC):
                bs = {p[0] for p in percore[c] if p is not None}
                if len(bs) > 1:
                    ok = False
            classes.append((L, percore, np_))
        if ok:
            break
    assert ok, "schedule fallback failed"

    # W group-block dedupe: pairs are processed in groups of 2 (two swapped
    # projection passes [qA|kB], [kA|qB] so both operands of the scores
    # matmul share a base partition).  A group block can be reused if on
    # every core the (headA, headB) at that block matches (or is pad).
    block_heads = [dict() for _ in range(NC)]   # per core: block -> (hA, hB)
    nblocks = 0
    out_classes = []
    ht_off = vv_base = qk_off = 0
    for L, percore, np_ in classes:
        nk = -(-L // KCH)
        Lp = nk * KCH
        ngrp = -(-np_ // 2)
        cls = _Cls()
        cls.L, cls.Lp, cls.nk, cls.np_, cls.ngrp = L, Lp, nk, np_, ngrp
        cls.pairs_percore = percore
        cls.ht_off = ht_off
        cls.vv_base = vv_base
        cls.qk_offs = []
        cls.w_blocks = []

        def _grp_heads(c, g):
            pa = percore[c][2 * g]
            pb = percore[c][2 * g + 1] if 2 * g + 1 < np_ else None
            return (pa[1] if pa is not None else None,
                    pb[1] if pb is not None else None)

        for g in range(ngrp):
            hs = [_grp_heads(c, g) for c in range(NC)]

            def _compat(cand):
                for c, (ha, hb) in enumerate(hs):
                    cur = block_heads[c].get(cand)
                    if cur is not None:
                        if ha is not None and cur[0] is not None and cur[0] != ha:
                            return False
                        if hb is not None and cur[1] is not None and cur[1] != hb:
                            return False
                return True

            beta = None
            for cand in range(nblocks):
                if _compat(cand):
                    beta = cand
                    break
            if beta is None:
                beta = nblocks
                nblocks += 1
            for c, (ha, hb) in enumerate(hs):
                cur = block_heads[c].get(beta, (None, None))
                block_heads[c][beta] = (ha if ha is not None else cur[0],
                                        hb if hb is not None else cur[1])
            cls.w_blocks.append(beta)
            cls.qk_offs.append((qk_off, qk_off + Lp))
            qk_off += 2 * Lp
        # v-proj streams the class's blocks as one range: need contiguity
        b0 = cls.w_blocks[0]
        if cls.w_blocks != list(range(b0, b0 + ngrp)):
            cls.w_blocks = list(range(nblocks, nblocks + ngrp))
            nblocks += ngrp
            for g in range(ngrp):
                for c in range(NC):
                    block_heads[c][cls.w_blocks[g]] = _grp_heads(c, g)
        cls.qslots = []
        q = 0
        while q < L:
            cls.qslots.append((q, min(QCH, L - q)))
            q += QCH
        ht_off += Lp
        vv_base += nk * np_
        out_classes.append(cls)

    slots = []
    eb_col = out_off = 0
    for ci, cls in enumerate(out_classes):
        for pl in range(cls.np_):
            for (qoff, qlen) in cls.qslots:
                sl = _Slot()
                sl.ci, sl.pl, sl.qoff, sl.qlen = ci, pl, qoff, qlen
                sl.nsub = -(-qlen // 128)
                sl.eb_col = eb_col
                sl.out_off = out_off
                eb_col += cls.nk * qlen
                out_off += sl.nsub
                slots.append(sl)

    sched = {
        "classes": out_classes,
        "slots": slots,
        "NW": nblocks,
        "HT_COLS": ht_off,
        "QKT_COLS": qk_off,
        "VCH": vv_base,
        "EB_COLS": eb_col,
        "NSUB": out_off,
        "lens": lens,
    }
    return sched


def _get_sched(lens_key):
    key = ("sched", lens_key)
    if key not in _CACHE:
        _CACHE[key] = _build_schedule(list(lens_key))
    return _CACHE[key]


# --------------------------------------------------------------------------
# device program
# --------------------------------------------------------------------------

def _build_nc(sched, use_bias):
    import concourse.mybir as mybir
    import concourse.tile as tile
    from concourse import bacc

    f16 = mybir.dt.float16

    nc = bacc.Bacc("TRN2", debug=False, num_devices=NC)
    # partition-major layouts: row p holds partition p's data contiguously
    # (large DMA descriptors). hT/W carry an extra 129th row for the bias.
    hT = nc.dram_tensor("hT", [129, 6 * sched["HT_COLS"]], f16,
                        kind="ExternalInput").ap()
    W = nc.dram_tensor("W", [129, 6 * sched["NW"] * 384], f16,
                       kind="ExternalInput").ap()
    Eb = nc.dram_tensor("Eb", [128, sched["EB_COLS"]], f16,
                        kind="ExternalInput").ap()
    out = nc.dram_tensor("out", [128, sched["NSUB"] * 65], f16,
                         kind="ExternalOutput").ap()

    with tile.TileContext(nc) as tc:
        _emit_body(nc, tc, tile, mybir, hT, W, Eb, out, sched, use_bias)
    nc.compile()
    return nc


def _emit_body(nc, tc, tile, mybir, hT, W, Eb, out, sched, use_bias):
    f16 = mybir.dt.float16
    f32 = mybir.dt.float32
    Exp = mybir.ActivationFunctionType.Exp
    classes = sched["classes"]
    slots = sched["slots"]
    NW = sched["NW"]

    with (
        tc.tile_pool(name="per", bufs=1) as per,
        tc.tile_pool(name="ebp", bufs=3) as ebp,
        tc.tile_pool(name="ste", bufs=3) as ste,
        tc.tile_pool(name="stp", bufs=9) as stp,
        tc.tile_pool(name="osb", bufs=3) as osb,
        tc.tile_pool(name="psc", bufs=2, space="PSUM") as psc,
        tc.tile_pool(name="pat", bufs=2, space="PSUM") as pat,
        tc.tile_pool(name="pj", bufs=2, space="PSUM") as pj,
    ):
        # ---- persistent tiles ------------------------------------------
        # W group block (384 cols): [qA|kB][kA|qB][vA|vB]
        hT_sb = per.tile([128, 6, sched["HT_COLS"]], f16)
        W_sb = per.tile([128, 6, NW, 384], f16)
        qkT = per.tile([128, sched["QKT_COLS"]], f16)
        vv = per.tile([128, sched["VCH"], 65], f16)
        nc.vector.memset(vv[:, :, 64:65], 1.0)
        HTC = sched["HT_COLS"]
        WC = NW * 384
        if use_bias:
            hT_last = per.tile([1, 6, HTC], f16)
            W_last2 = per.tile([1, 6, NW, 384], f16)
            nc.sync.dma_start(
                hT_last, hT[128:129, :].rearrange("o (i c) -> o i c", c=HTC)
            )
            nc.sync.dma_start(
                W_last2,
                W[128:129, :].rearrange("o (i w k) -> o i w k", i=6, k=384),
            )

        # ---- upfront DMAs (class 0 + W first, per-ic for fast start) ----
        def dma_ht(cls, ic):
            c0 = cls.ht_off
            nc.sync.dma_start(
                hT_sb[:, ic, c0:c0 + cls.Lp],
                hT[0:128, ic * HTC + c0:ic * HTC + c0 + cls.Lp],
            )

        for ic in range(6):
            dma_ht(classes[0], ic)
            nc.sync.dma_start(
                W_sb[:, ic].rearrange("p w k -> p (w k)"),
                W[0:128, ic * WC:(ic + 1) * WC],
            )
        for cls in classes[1:]:
            for ic in range(6):
                dma_ht(cls, ic)

        # ---- projection job closures -----------------------------------
        def qk_job(cls, g, pss, lc):
            # pss 0: W cols [0:128] = [qA|kB]; pss 1: [128:256] = [kA|qB]
            def run():
                cw = min(QCH, cls.Lp - lc * QCH)
                beta = cls.w_blocks[g]
                c0 = 128 * pss
                dst = cls.qk_offs[g][pss] + lc * QCH
                ps = pj.tile([128, QCH], f32, tag="pj", name="ps_qk")
                for ic in range(6):
                    nc.tensor.matmul(
                        ps[:, :cw],
                        W_sb[:, ic, beta, c0:c0 + 128],
                        hT_sb[:, ic, cls.ht_off + lc * QCH:
                              cls.ht_off + lc * QCH + cw],
                        start=(ic == 0), stop=(ic == 5 and not use_bias),
                    )
                if use_bias:
                    nc.tensor.matmul(
                        ps[:, :cw],
                        W_last2[:, 0, beta, c0:c0 + 128],
                        hT_last[:, 0, cls.ht_off + lc * QCH:
                                cls.ht_off + lc * QCH + cw],
                        start=False, stop=True,
                    )
                nc.vector.tensor_copy(qkT[:, dst:dst + cw], ps[:, :cw])
            return run

        def v_job(cls, kc):
            # moving = v cols of the class's group blocks; the last group's
            # second half is skipped when np_ is odd (two matmul chains)
            def run():
                npr = cls.np_
                b0 = cls.w_blocks[0]
                nfull = npr // 2
                ps = pj.tile([128, QCH], f32, tag="pj", name="ps_v")
                segs = []
                if nfull:
                    segs.append((0, W_sb[:, :, b0:b0 + nfull, 256:384],
                                 nfull * 128,
                                 (W_last2[:, 0, b0:b0 + nfull, 256:384]
                                  if use_bias else None)))
                if npr % 2:
                    gl = b0 + nfull
                    segs.append((nfull * 128, W_sb[:, :, gl, 256:320], 64,
                                 (W_last2[:, 0, gl, 256:320]
                                  if use_bias else None)))
                for (o0, wap, n, wlast) in segs:
                    for ic in range(6):
                        nc.tensor.matmul(
                            ps[:, o0:o0 + n],
                            hT_sb[:, ic, cls.ht_off + kc * KCH:
                                  cls.ht_off + (kc + 1) * KCH],
                            wap[:, ic],
                            start=(ic == 0), stop=(ic == 5 and not use_bias),
                        )
                    if use_bias:
                        nc.tensor.matmul(
                            ps[:, o0:o0 + n],
                            hT_last[:, 0, cls.ht_off + kc * KCH:
                                    cls.ht_off + (kc + 1) * KCH],
                            wlast,
                            start=False, stop=True,
                        )
                nc.vector.tensor_copy(
                    vv[:, cls.vv_base + kc * npr:
                       cls.vv_base + (kc + 1) * npr, 0:64],
                    ps[:, :npr * 64].rearrange("p (h d) -> p h d", d=64),
                )
            return run

        def proj_jobs(ci):
            cls = classes[ci]
            jobs = []
            for g in range(cls.ngrp):
                for pss in range(2):
                    for lc in range(-(-cls.Lp // QCH)):
                        jobs.append(qk_job(cls, g, pss, lc))
            for kc in range(cls.nk):
                jobs.append(v_job(cls, kc))
            return jobs

        # ---- Eb prefetch ------------------------------------------------
        eb_tiles = {}

        def issue_eb(si):
            sl = slots[si]
            cls = classes[sl.ci]
            t = ebp.tile([128, 8, QCH], f16, tag="eb", name="eb")
            eb_tiles[si] = t
            nk1 = cls.nk // 2
            for (k0, k1) in ((0, nk1), (nk1, cls.nk)):
                if k1 <= k0:
                    continue
                c0 = sl.eb_col + k0 * sl.qlen
                nc.sync.dma_start(
                    t[:, k0:k1, :sl.qlen],
                    Eb[:, c0:c0 + (k1 - k0) * sl.qlen].rearrange(
                        "p (n q) -> p n q", q=sl.qlen
                    ),
                )

        # ---- prologue ---------------------------------------------------
        for job in proj_jobs(0):
            job()
        issue_eb(0)
        if len(slots) > 1:
            issue_eb(1)

        # ---- main loop --------------------------------------------------
        # software pipeline: scores/exp/mul for slot i run while the PV
        # matmuls for slot i-1 stream (each q-sub's PSUM accumulation group
        # is sequential in its own bank: start=True resets the whole bank)
        def emit_pv_all(pend):
            (pts, cls_p, pl_p, qlen_p, nsub_p, out_off_p) = pend
            ob = osb.tile([128, 4, 65], f16, tag="ob", name="ob")
            for sub in range(nsub_p):
                qn = min(128, qlen_p - sub * 128)
                att = pat.tile([128, 128], f32, tag="att", name="att",
                               padded_shape=[128, QCH])
                for kc in range(cls_p.nk):
                    nc.tensor.matmul(
                        att[0:qn, 0:65],
                        pts[kc // 2][:, kc % 2, sub * 128:sub * 128 + qn],
                        vv[:, cls_p.vv_base + kc * cls_p.np_ + pl_p, :],
                        start=(kc == 0), stop=(kc == cls_p.nk - 1),
                    )
                nc.vector.tensor_copy(ob[:, sub, :], att[:, 0:65])
            nc.sync.dma_start(
                out[:, out_off_p * 65:(out_off_p + nsub_p) * 65].rearrange(
                    "p (n x) -> p n x", x=65
                ),
                ob[:, :nsub_p, :],
            )

        si = 0
        pending = None
        for ci, cls in enumerate(classes):
            fillers = proj_jobs(ci + 1) if ci + 1 < len(classes) else []
            n_iters = cls.np_ * len(cls.qslots) * cls.nk
            stride = max(1, n_iters // max(1, len(fillers)))
            it = 0
            fi = 0
            for pl in range(cls.np_):
                for (qoff, qlen) in cls.qslots:
                    sl = slots[si]
                    eb = eb_tiles.pop(si)
                    g, half = pl // 2, pl % 2
                    off1, off2 = cls.qk_offs[g]
                    # half 0: q in T1[0:64],  k in T2[0:64]
                    # half 1: q in T2[64:128], k in T1[64:128]
                    p0 = 64 * half
                    koff = off1 if half else off2
                    qoff_t = off2 if half else off1

                    pts = []
                    for kg in range(0, cls.nk, 2):
                        n2 = min(2, cls.nk - kg)
                        sps = psc.tile([128, 2, QCH], f32, tag="sc",
                                       name="sps")
                        for j in range(n2):
                            kc = kg + j
                            nc.tensor.matmul(
                                sps[:, j, :qlen],
                                qkT[p0:p0 + 64,
                                    koff + kc * KCH:koff + (kc + 1) * KCH],
                                qkT[p0:p0 + 64,
                                    qoff_t + qoff:qoff_t + qoff + qlen],
                                start=True, stop=True,
                            )
                            if (fillers and fi < len(fillers)
                                    and it % stride == 0):
                                fillers[fi]()
                                fi += 1
                            it += 1
                        es = ste.tile([128, 2, QCH], f16, tag="es", name="es")
                        nc.scalar.activation(
                            es[:, :n2, :qlen], sps[:, :n2, :qlen], Exp
                        )
                        pt = stp.tile([128, 2, QCH], f16, tag="pt", name="pt")
                        mul_eng = nc.gpsimd if (kg // 2) % 4 == 3 else nc.vector
                        mul_eng.tensor_mul(
                            pt[:, :n2, :qlen], es[:, :n2, :qlen],
                            eb[:, kg:kg + n2, :qlen]
                        )
                        pts.append(pt)
                    if pending is not None:
                        emit_pv_all(pending)
                    pending = (pts, cls, pl, qlen, sl.nsub, sl.out_off)
                    if si + 2 < len(slots):
                        issue_eb(si + 2)
                    si += 1
            while fi < len(fillers):
                fillers[fi]()
                fi += 1
        emit_pv_all(pending)


def _get_nc(lens_key, use_bias):
    key = ("nc", lens_key, use_bias)
    if key not in _CACHE:
        _CACHE[key] = _build_nc(_get_sched(lens_key), use_bias)
    return _CACHE[key]


# --------------------------------------------------------------------------
# host pack / unpack
# --------------------------------------------------------------------------

def prepare_in_maps(inputs):
    hidden = np.asarray(inputs["hidden_states"], np.float32)
    Wf = np.asarray(inputs["Wqkv_w"], np.float32)
    bvec = np.asarray(inputs["Wqkv_b"], np.float32)
    bias = np.asarray(inputs["bias"], np.float32)
    indices = np.asarray(inputs["indices"], np.int32)
    cu = np.asarray(inputs["cu_seqlens"], np.int64)
    lens = np.diff(cu).astype(np.int64)
    nnz = hidden.shape[0]

    # valid tokens must be the first L of each batch row-block
    expect = np.concatenate(
        [b * S + np.arange(l) for b, l in enumerate(lens)]
    ) if len(lens) == B else None
    contiguous = (
        expect is not None
        and indices.shape[0] == expect.shape[0]
        and np.array_equal(indices, expect)
    )
    if not contiguous:
        # fallback: dense full-length processing, scatter rows
        lens = np.full(B, S, np.int64)
        hp = np.zeros((B * S, HID), np.float32)
        hp[indices] = hidden
        tok = [hp[b * S:(b + 1) * S] for b in range(B)]
    else:
        tok = [hidden[cu[b]:cu[b + 1]] for b in range(B)]

    lens_key = tuple(int(x) for x in lens)
    sched = _get_sched(lens_key)
    use_bias = bool(np.any(bvec != 0.0))

    Ws = Wf.copy()
    Ws[:, :HID] *= 0.125          # fold 1/sqrt(D) into q
    bs = bvec.copy()
    bs[:HID] *= 0.125

    classes = sched["classes"]
    slots = sched["slots"]
    NW = sched["NW"]

    HTC = sched["HT_COLS"]

    def prep_core(c):
        hTa = np.zeros((HID + 1, HTC), np.float16)
        hTa[HID] = 1.0
        Wd = np.zeros((HID + 1, NW * 384), np.float16)
        Ebd = np.zeros((128, sched["EB_COLS"]), np.float16)
        for cls in classes:
            batches = {p[0] for p in cls.pairs_percore[c] if p is not None}
            if batches:
                b0 = next(iter(batches))
                L = int(lens[b0])
                hTa[0:HID, cls.ht_off:cls.ht_off + L] = tok[b0].T
            for pl, pair in enumerate(cls.pairs_percore[c]):
                if pair is None:
                    continue
                _, h = pair
                beta = cls.w_blocks[pl // 2]
                half = pl % 2
                # group block: [qA|kB][kA|qB][vA|vB]
                qc0 = beta * 384 + (192 if half else 0)
                kc0 = beta * 384 + (64 if half else 128)
                vc0 = beta * 384 + (320 if half else 256)
                Wd[0:HID, qc0:qc0 + 64] = Ws[:, h * D:(h + 1) * D]
                Wd[0:HID, kc0:kc0 + 64] = Ws[:, HID + h * D:HID + (h + 1) * D]
                Wd[0:HID, vc0:vc0 + 64] = \
                    Ws[:, 2 * HID + h * D:2 * HID + (h + 1) * D]
                Wd[HID, qc0:qc0 + 64] = bs[h * D:(h + 1) * D]
                Wd[HID, kc0:kc0 + 64] = bs[HID + h * D:HID + (h + 1) * D]
                Wd[HID, vc0:vc0 + 64] = \
                    bs[2 * HID + h * D:2 * HID + (h + 1) * D]
        with np.errstate(under="ignore"):
            for sl in slots:
                cls = classes[sl.ci]
                pair = cls.pairs_percore[c][sl.pl]
                if pair is None:
                    continue
                b, h = pair
                L = int(lens[b])
                sub = bias[b, h, sl.qoff:sl.qoff + sl.qlen, 0:L]
                arr = np.zeros((cls.nk * KCH, sl.qlen), np.float16)
                arr[:L] = (np.exp(sub) * EB_SCALE).T.astype(np.float16)
                Ebd[:, sl.eb_col:sl.eb_col + cls.nk * sl.qlen] = (
                    arr.reshape(cls.nk, 128, sl.qlen)
                    .transpose(1, 0, 2).reshape(128, cls.nk * sl.qlen)
                )
        # partition-major repack: row p holds its 6 ic chunks contiguously
        hT_pm = np.zeros((129, 6 * HTC), np.float16)
        hT_pm[0:128] = (hTa[0:HID].reshape(6, 128, HTC)
                        .transpose(1, 0, 2).reshape(128, 6 * HTC))
        hT_pm[128, 0:HTC] = hTa[HID]
        W_pm = np.zeros((129, 6 * NW * 384), np.float16)
        W_pm[0:128] = (Wd[0:HID].reshape(6, 128, NW * 384)
                       .transpose(1, 0, 2).reshape(128, 6 * NW * 384))
        W_pm[128, 0:NW * 384] = Wd[HID]
        return {"hT": hT_pm, "W": W_pm, "Eb": Ebd}

    from concurrent.futures import ThreadPoolExecutor
    with ThreadPoolExecutor(max_workers=8) as ex:
        in_maps = list(ex.map(prep_core, range(NC)))

    meta = {
        "lens_key": lens_key,
        "cu": cu,
        "nnz": nnz,
        "contiguous": contiguous,
        "indices": indices,
    }
    return in_maps, meta, use_bias


def postprocess(results, meta):
    sched = _get_sched(meta["lens_key"])
    classes = sched["classes"]
    slots = sched["slots"]
    cu = meta["cu"]
    if meta["contiguous"]:
        out_full = np.zeros((meta["nnz"], HID), np.float32)
    else:
        out_full = np.zeros((B * S, HID), np.float32)
    for c in range(NC):
        o = np.asarray(results[c]["out"], np.float32)   # [128, NSUB*65]
        o = o.reshape(128, sched["NSUB"], 65).transpose(1, 0, 2)
        for sl in slots:
            cls = classes[sl.ci]
            pair = cls.pairs_percore[c][sl.pl]
            if pair is None:
                continue
            b, h = pair
            base = (cu[b] if meta["contiguous"] else b * S)
            for sub in range(sl.nsub):
                qn = min(128, sl.qlen - sub * 128)
                blk = o[sl.out_off + sub, :qn]
                att = blk[:, :64] / blk[:, 64:65]
                r0 = base + sl.qoff + sub * 128
                out_full[r0:r0 + qn, h * D:(h + 1) * D] = att
    if not meta["contiguous"]:
        out_full = out_full[meta["indices"]]
    return out_full


def _run_spmd(in_maps, meta, use_bias=True, trace=False):
    from concourse.bass_utils import run_bass_kernel_spmd
    return run_bass_kernel_spmd(
        _get_nc(meta["lens_key"], use_bias), in_maps,
        core_ids=list(range(NC)), trace=trace,
    )


def kernel(**inputs):
    in_maps, meta, use_bias = prepare_in_maps(inputs)
    res = _run_spmd(in_maps, meta, use_bias=use_bias)
    return postprocess(res.results, meta)
